# revision 52
# baseline (speedup 1.0000x reference)
"""Trainium2 (8 NeuronCores) kernel for single-head causal attention.

Problem: x [8, 2048, 1024] f32; Wq/Wk/Wv [1024, 128] f32.
    q = x @ Wq ; k = x @ Wk ; v = x @ Wv          (per batch row)
    out = softmax(causal(q @ k^T / sqrt(128))) @ v  -> [8, 2048, 128] f32

Sharding: pure data-parallel — one batch row per NeuronCore, weights
replicated. No collectives.

Per-core algorithm (bf16 matmul inputs, f32 PSUM accumulation):
  Host supplies xT = x[b].T  [D, T] in bf16 (layout prep only).
  A) qT/kT [H=128 part, T] with W-chunks stationary over 8 D-chunks;
     v [T-block part, H] computed per k-block inside phase B. An all-ones
     column is appended to v so the softmax denominator falls out of the
     output matmul for free.
  B) Scores computed TRANSPOSED: sT[k,q] = kT_j-block-stationary @ qT,
     exact-causal (q in [j*128, T) per k-block j). exp(scale*s) runs on
     ScalarE straight out of PSUM into a CAUSAL-PACKED bf16 wT tile.
     No max-subtraction: |scale*s| <= ~7 here, safely in f32/bf16 range.
     Diagonal 128x128 blocks additionally get a multiplicative 0/1
     strictly-causal mask into separate diag tiles.
  C) out[q,h] accumulates over k-blocks j<=i with wT blocks stationary and
     v_aug moving (N=129). Column 128 of PSUM is l = sum_k w; normalize
     with one reciprocal + ScalarE copy-with-per-partition-scale, DMA out.

Input streaming (the v1 kernel lost ~7 us here): per-DMA throughput caps
well below the ~358 GB/s aggregate (full-width chunk solo ~92 GB/s), and
concurrent DMAs round-robin at packet granularity so with all 8 chunk
DMAs in flight chunk 0's completion fires only after ~60% of the input
transferred. v4: every chunk is partition-split into sub-DMAs across the
SP+ACT trigger rings, ~3 chunks in flight via Pool WAR-link chaining, so
completions arrive in consumption order at full aggregate bandwidth.

PE warm-up: HAM's clock governor ramps with sustained engine density
(~1.2 GHz at start, full clock after ~3.4 us of UNBROKEN work; gaps delay
the ramp and the whole kernel runs slow). Dummy matmuls bridge the
launch/DMA dead window so the PE never idles from first dispatch on.

Engine discipline: hardware compute instructions carry at most ONE
semaphore wait (bacc's legalization splits the rest into event-semaphore
junctions; redundant same-engine self-waits are stripped post-build —
PE/ACT/DVE complete strictly in order). Tile tracks dependencies at
subtile granularity. exp and the final per-row scale run on ScalarE;
per-partition-scalar multiplies must use ScalarE activation scale APs
(DVE tensor_scalar AP / stride-0 broadcasts give wrong results on
hardware). The A->B junction interleaves q/k stop-matmuls with half-tile
PSUM->SBUF copies (qT halves on DVE, kT halves on ACT) so the first score
matmul's deps are ready the cycle the last stop-matmul retires.
"""

from contextlib import ExitStack

import ml_dtypes
import numpy as np

B, T, D, H = 8, 2048, 1024, 128
P = 128
DC = D // P  # 8 contraction chunks
TB = T // P  # 16 token blocks
QG = T // 512  # 4 512-wide token groups
SCALE = 1.0 / float(np.sqrt(H))

_CACHE = {}
LAST_RESULT = None


def _build():
    import concourse.bacc as bacc
    import concourse.mybir as mybir
    import concourse.tile as tile

    f32 = mybir.dt.float32
    bf16 = mybir.dt.bfloat16
    EXP = mybir.ActivationFunctionType.Exp
    MULT = mybir.AluOpType.mult
    ADD = mybir.AluOpType.add

    nc = bacc.Bacc()
    xT_h = nc.declare_dram_parameter("xT", [D, T], bf16, isOutput=False)
    # weights host-prelayouted to [p, c, h] so the DMA is contiguous
    # (2048 B per partition row; the [D, H] layout gave 256 B strided rows
    # below the 512 B DMA granule -> RMW-penalized, hogging the input queue)
    wq_h = nc.declare_dram_parameter("Wq", [P, DC, H], bf16, isOutput=False)
    wk_h = nc.declare_dram_parameter("Wk", [P, DC, H], bf16, isOutput=False)
    wv_h = nc.declare_dram_parameter("Wv", [P, DC, H], bf16, isOutput=False)
    mask_h = nc.declare_dram_parameter("mask", [P, P], bf16, isOutput=False)
    out_h = nc.declare_dram_parameter("out", [T, H], f32, isOutput=True)

    with tile.TileContext(nc) as tc:
        with ExitStack() as ctx:
            singles = ctx.enter_context(tc.tile_pool(name="singles", bufs=1))

            xT_sb = singles.tile([P, DC, T], bf16)
            wq_sb = singles.tile([P, DC, H], bf16)
            wk_sb = singles.tile([P, DC, H], bf16)
            wv_sb = singles.tile([P, DC, H], bf16)
            mask_sb = singles.tile([P, P], bf16)
            mask2_sb = singles.tile([P, P], bf16)
            qT_sb = singles.tile([P, T], bf16)
            kT_sb = singles.tile([P, T], bf16)
            v_sb = singles.tile([P, TB, 132], bf16)  # [...,128] = ones col
            wT_sb = singles.tile([P, 17408], bf16)  # causal-packed
            dw_sb = singles.tile([P, TB, P], bf16)  # masked diagonal blocks
            link_sb = singles.tile([P, DC + 2], bf16)  # DMA-chain dummies
            warm_sb = singles.tile([P, 512], bf16)
            # per-iteration epilogue slices (no pool recycling -> no WAR waits)
            rec_all = singles.tile([P, TB], f32)
            os_all = singles.tile([P, TB, P], f32)  # unnormalized staging
            ot_all = singles.tile([P, TB, P], f32)

            # Pool engine starts earliest (~6.0 us; its preamble memsets are
            # the first real ops in the v1 trace) — it seeds the warm-up
            # fodder the PE's dummy matmuls read.
            nc.gpsimd.memset(warm_sb, 0.0)
            nc.vector.memset(v_sb[:, :, 128:129], 1.0)

            # --- Input stream. Measured DMA law (see transcript probes): a
            # single full-width [128, 2048] DMA sustains only ~92 GB/s; 4
            # partition-subs of one chunk ~143; 4 concurrent full chunks
            # ~278; 8 concurrent ~333 (aggregate cap ~358). Per-DMA
            # throughput is the wall, so EVERY chunk is split into
            # partition-subs, and ~3 chunks (6 subs) stay in flight: chunk c
            # (c>=3) WAR-waits a Pool "link" op that RAW-waits chunk c-3's
            # completion. Completions then arrive IN CONSUMPTION ORDER every
            # ~1.4 us (all-concurrent DMAs round-robin at packet granularity,
            # which lands every completion at ~60% of the whole transfer —
            # v1 idled the PE 7.5 us waiting for chunk 0). Chunk 0 gets 4
            # subs (~143 GB/s solo) since its completion opens phase A.
            xT_ap = xT_h[:]

            def chunk_dma(c, nsub):
                w = P // nsub
                for s in range(nsub):
                    eng = nc.sync if s % 2 == 0 else nc.scalar
                    eng.dma_start(
                        out=xT_sb[w * s : w * (s + 1), c, :],
                        in_=xT_ap[c * P + w * s : c * P + w * (s + 1), :],
                    )

            def link(read_done_ap, war_ap, li):
                # RAW on read_done_ap's producer; the next DMA (writing
                # war_ap's region) then WARs on this op -> its trigger fires
                # only after the producer's completion semaphore.
                nc.gpsimd.tensor_tensor(
                    link_sb[:, li : li + 1], read_done_ap, war_ap, ADD
                )

            # Chunk 0 as a 4-sub head start (~143 GB/s vs 92 full-width solo)
            # + Wq/Wk as 2-subs: phase A's gate lands ~13 us. Chunks 1-7 ride
            # FULL-WIDTH and UNCHAINED right behind: concurrent full-width
            # DMAs each sustain near their solo rate (aggregate ~330), so
            # completions arrive in trigger order every ~0.65 us — faster
            # than the PE's 1.76 us/chunk consumption. (Chained or
            # partition-split streams cap at ~210-240 GB/s aggregate; v4's
            # chained 2-sub pipeline ran a 2.3 us chunk pitch and starved
            # the PE through all of phase A.)
            chunk_dma(0, 4)
            nc.sync.dma_start(out=wq_sb[0:64], in_=wq_h[0:64])
            nc.scalar.dma_start(out=wq_sb[64:128], in_=wq_h[64:128])
            nc.sync.dma_start(out=wk_sb[0:64], in_=wk_h[0:64])
            nc.scalar.dma_start(out=wk_sb[64:128], in_=wk_h[64:128])
            for c in range(1, DC):
                eng = nc.sync if c % 2 == 1 else nc.scalar
                eng.dma_start(out=xT_sb[:, c, :], in_=xT_ap[c * P : (c + 1) * P, :])
            # Phase-B-only inputs trail the stream (needed ~25 us) so they
            # don't steal chunk bandwidth.
            link(xT_sb[:, 5, T - 2 : T - 1], wv_sb[:, 0, 0:1], 0)
            nc.sync.dma_start(out=wv_sb, in_=wv_h[:])
            link(xT_sb[:, 6, T - 2 : T - 1], mask_sb[:, 0:1], 1)
            nc.scalar.dma_start(out=mask_sb, in_=mask_h[:])
            # ACT pre-touch: moves the mask's DMA wait onto a junction copy so
            # the per-j diag multiply's two deps (exp + mask) merge into one
            # ACT wait — hardware instructions carry at most one sem wait.
            nc.scalar.copy(mask2_sb, mask_sb)

            # --- Phase A1: q/k projections, d-chunk OUTER so each xT chunk is
            # consumed as its DMA lands (PE overlaps the input load). 8 PSUM
            # banks live at once; pool scoped so phase B/C reuse the space.
            with tc.tile_pool(name="psQK", bufs=1, space="PSUM") as psQK:
                # PSUM dep-tracking is TILE-granular: a copy reading one
                # 512-group of a 4-bank accumulator WARs every later stop
                # matmul writing ANY group of that tile (v8 lost ~2.3 us to
                # exactly that). One single-bank tile per 512-group instead:
                # stops and copies of different groups never interact.
                qps = [
                    psQK.tile([P, 512], f32, tag=f"qps{g}", name=f"qps{g}")
                    for g in range(QG)
                ]
                kps = [
                    psQK.tile([P, 512], f32, tag=f"kps{g}", name=f"kps{g}")
                    for g in range(QG)
                ]
                # PE warm-up fodder: the PE sequencer starts ~8.1 us (its
                # ~770-instruction queue loads last); 12 x 512 dummies end
                # ~13.2 us, dovetailing with chunk 0's completion. HAM's
                # clock governor ramps with engine DENSITY — v3's sparse
                # early stream delayed full clock to 25.7 us and the WHOLE
                # kernel ran ~15% slow — so the PE must never idle from
                # warm-up start onward. They write qps[0] BEFORE its real
                # accumulation group begins (start=True clears the bank).
                for _ in range(13):
                    nc.tensor.matmul(
                        qps[0], warm_sb[:, 0:128], warm_sb,
                        start=True, stop=True,
                    )

                def qk_mm(is_q, g, c):
                    w_sb = wq_sb if is_q else wk_sb
                    acc = qps[g] if is_q else kps[g]
                    nc.tensor.matmul(
                        acc,
                        w_sb[:, c, :],
                        xT_sb[:, c, g * 512 : (g + 1) * 512],
                        start=(c == 0),
                        stop=(c == DC - 1),
                    )

                for c in range(DC - 1):
                    for is_q in (True, False):
                        for g in range(QG):
                            qk_mm(is_q, g, c)
                # Last chunk: interleave each 512-wide q/k stop-matmul with
                # ITS PSUM->SBUF copy (qT copies on DVE, kT on ACT). With
                # per-group tiles the copies carry no WAR against the other
                # groups' stops, so the PE runs all 8 stops back-to-back and
                # the last copy — which gates the B-pools' alloc via the
                # psQK release — trails the last stop by only ~450 ns.
                for g in range(QG):
                    qk_mm(True, g, DC - 1)
                    nc.vector.tensor_copy(
                        qT_sb[:, g * 512 : (g + 1) * 512], qps[g]
                    )
                    qk_mm(False, g, DC - 1)
                    nc.scalar.copy(
                        kT_sb[:, g * 512 : (g + 1) * 512], kps[g]
                    )

            with ExitStack() as ctx2:
                psS = ctx2.enter_context(
                    tc.tile_pool(name="psS", bufs=2, space="PSUM")
                )
                psV = ctx2.enter_context(
                    tc.tile_pool(name="psV", bufs=1, space="PSUM")
                )
                psO = ctx2.enter_context(
                    tc.tile_pool(name="psO", bufs=3, space="PSUM")
                )

                # --- Phases B+A2+C, software-pipelined by one j: per k-block
                # j emit its score matmuls + exp + v projection, then output
                # group C_{j-1}, whose inputs (exps/dw/v for blocks <= j-1)
                # are all complete by then — so C's matmuls carry no waits and
                # the PE stream stays dense while ScalarE exps run alongside.
                out_ap = out_h[:]

                # Causal-packed wT layout: segment for k-block j holds
                # q in [j*128, T) at packed offset OFF[j]; segments are
                # back-to-back so exp runs in maximal 1024-wide ops across
                # block boundaries (ACT op overhead is ~352 cycles each).
                OFF = [0] * (TB + 1)
                for j in range(TB):
                    OFF[j + 1] = OFF[j] + (T - j * P)
                TOTAL = OFF[TB]  # 17408

                def wT_at(jj, qstart, width):
                    o = OFF[jj] + (qstart - jj * P)
                    return wT_sb[:, o : o + width]

                def emit_c_group(i):
                    po = psO.tile([P, 132], f32, tag="psO", name=f"po{i}")
                    for jj in range(i):
                        nc.tensor.matmul(
                            po[:, 0:129],
                            wT_at(jj, i * P, P),
                            v_sb[:, jj, 0:129],
                            start=(jj == 0),
                            stop=False,
                        )
                    nc.tensor.matmul(
                        po[:, 0:129],
                        dw_sb[:, i, :],
                        v_sb[:, i, 0:129],
                        start=(i == 0),
                        stop=True,
                    )
                    nc.vector.reciprocal(rec_all[:, i : i + 1], po[:, 128:129])
                    # per-partition normalize OFF ScalarE: with it there,
                    # ACT's B-phase ledger (17 exps ~17us + 16 muls ~7us)
                    # exceeded the PE's ~22us window and became co-critical.
                    # DVE's tensor_scalar reads PSUM WRONG on hardware (rel
                    # err 19!) but is correct from SBUF (probe), and Pool's
                    # tensor_scalar takes 2us/op (slow DSP): so DVE copies
                    # PSUM->SBUF (its usual, correct path), then multiplies
                    # in SBUF, both on DVE.
                    # The LAST two groups normalize on ScalarE directly from
                    # PSUM (its exps are done by then; the activation-scale
                    # path is PSUM-correct) — one hop fewer on the tail.
                    if i >= TB - 2:
                        nc.scalar.mul(
                            ot_all[:, i, :], po[:, 0:H], rec_all[:, i : i + 1]
                        )
                    else:
                        nc.vector.tensor_copy(os_all[:, i, :], po[:, 0:H])
                        nc.vector.tensor_scalar_mul(
                            ot_all[:, i, :], os_all[:, i, :], rec_all[:, i : i + 1]
                        )
                    # last group's DMA triggers from the ACT ring — same
                    # engine as its normalize, no cross-engine hop.
                    deng = nc.scalar if i == TB - 1 else nc.sync
                    deng.dma_start(
                        out=out_ap[i * P : (i + 1) * P, :], in_=ot_all[:, i, :]
                    )

                def emit_j_epilogue(j):
                    # diag mask (on Pool — all-SBUF, keeps DVE for the
                    # PSUM-side copies), v projection, pipelined output
                    # group. The LAST diag multiply sits on the kernel's
                    # critical tail: DVE's 190 ns beats Pool's 405 ns there.
                    eng = nc.vector if j == TB - 1 else nc.gpsimd
                    eng.tensor_tensor(
                        dw_sb[:, j, :], wT_at(j, j * P, P), mask2_sb, MULT
                    )
                    pv = psV.tile([P, H], f32, tag="psV")
                    for c in range(DC):
                        nc.tensor.matmul(
                            pv,
                            xT_sb[:, c, j * P : (j + 1) * P],
                            wv_sb[:, c, :],
                            start=(c == 0),
                            stop=(c == DC - 1),
                        )
                    nc.vector.tensor_copy(v_sb[:, j, 0:H], pv)
                    if j > 0:
                        emit_c_group(j - 1)

                # 1024-wide exp chunks amortize ACT's ~352-cycle fixed cost,
                # but the tail goes FINER: segs 13/14/15's diag heads all sit
                # in the last 1024 cols, so one last big exp would serialize
                # the last three output groups (~58 matmuls) behind it. Tail
                # chunks split exactly AT segment boundaries, so the final
                # exp is ONLY seg15's 128-col head — everything else the
                # last output group reads is exp'd before it.
                chunk_bounds = [(t, 1024) for t in range(0, 16384, 1024)]
                chunk_bounds += [(16384, 256), (16640, 384), (17024, 256), (17280, 128)]
                next_done = 0  # next j whose epilogue is pending
                for ts, tw in chunk_bounds:
                    ps = psS.tile([P, 1024], f32, tag="psS")
                    # score matmuls covering packed [ts, ts+tw): split at the
                    # PSUM bank boundary (ts+512) and at segment boundaries
                    for j in range(TB):
                        lo = max(ts, OFF[j])
                        hi = min(ts + tw, OFF[j + 1])
                        a = lo
                        while a < hi:
                            bank_end = ts + 512 if a < ts + 512 else ts + 1024
                            b = min(hi, bank_end)
                            qg = j * P + (a - OFF[j])
                            nc.tensor.matmul(
                                ps[:, a - ts : b - ts],
                                kT_sb[:, j * P : (j + 1) * P],
                                qT_sb[:, qg : qg + (b - a)],
                                start=True,
                                stop=True,
                            )
                            a = b
                    nc.scalar.activation(
                        wT_sb[:, ts : ts + tw], ps[:, :tw], EXP, scale=SCALE
                    )
                    # epilogue j needs only segment j's first 128 cols exp'd
                    # (dw_j's diag region; C_{j-1}'s deepest read is shallower)
                    while next_done < TB and OFF[next_done] + P <= ts + tw:
                        emit_j_epilogue(next_done)
                        next_done += 1
                emit_c_group(TB - 1)

    _strip_self_waits(nc)
    nc.finalize()  # Bacc.compile(): wait legalization + register allocation
    return nc


def _strip_self_waits(nc):
    """Drop same-engine semaphore waits on in-order engines (PE/ACT/DVE
    execute and complete strictly in order, so a self-wait is redundant).
    Tile emits them conservatively; walrus allows only one sem wait per
    compute instruction, and these push some matmuls/tensor-ops over."""
    prefixes = {"PE": "PE_", "Activation": "Activation_", "DVE": "DVE_"}
    for bb in nc.m.functions[0].blocks:
        for inst in bb.instructions:
            si = inst.sync_info
            if not si or not si.on_wait:
                continue
            pref = prefixes.get(str(inst.engine).split(".")[-1])
            if pref is None:
                continue
            keep = [w for w in si.on_wait if not (w.ant_name or "").startswith(pref)]
            if len(keep) != len(si.on_wait):
                si.on_wait = keep
                inst.sync_info = si


def kernel(**inputs):
    global LAST_RESULT
    x = np.asarray(inputs["x"], dtype=np.float32)
    bf = ml_dtypes.bfloat16
    w_bf = {
        k: np.ascontiguousarray(
            np.asarray(inputs[k], dtype=np.float32)
            .astype(bf)
            .reshape(DC, P, H)
            .transpose(1, 0, 2)
        )
        for k in ("Wq", "Wk", "Wv")
    }
    # dw[p=k_local, f=q_local] keeps entries with k <= q
    mask01 = (
        (np.arange(P)[:, None] <= np.arange(P)[None, :]).astype(np.float32).astype(bf)
    )

    if "nc" not in _CACHE:
        _CACHE["nc"] = _build()
    nc = _CACHE["nc"]

    from concourse.bass_utils import run_bass_kernel_spmd

    in_maps = [
        {
            "xT": np.ascontiguousarray(x[b].T).astype(bf),
            "Wq": w_bf["Wq"],
            "Wk": w_bf["Wk"],
            "Wv": w_bf["Wv"],
            "mask": mask01,
        }
        for b in range(B)
    ]
    res = run_bass_kernel_spmd(nc, in_maps, core_ids=list(range(B)))
    LAST_RESULT = res
    return np.stack([res.results[b]["out"] for b in range(B)]).astype(np.float32)


# revision 53
# speedup vs baseline: 1.0178x; 1.0178x over previous
"""Trainium2 (8 NeuronCores) kernel for single-head causal attention.

Problem: x [8, 2048, 1024] f32; Wq/Wk/Wv [1024, 128] f32.
    q = x @ Wq ; k = x @ Wk ; v = x @ Wv          (per batch row)
    out = softmax(causal(q @ k^T / sqrt(128))) @ v  -> [8, 2048, 128] f32

Sharding: pure data-parallel — one batch row per NeuronCore, weights
replicated. No collectives.

Per-core algorithm (bf16 matmul inputs, f32 PSUM accumulation):
  Host supplies xT = x[b].T  [D, T] in bf16 (layout prep only).
  A) qT/kT [H=128 part, T] with W-chunks stationary over 8 D-chunks;
     v [T-block part, H] computed per k-block inside phase B. An all-ones
     column is appended to v so the softmax denominator falls out of the
     output matmul for free.
  B) Scores computed TRANSPOSED: sT[k,q] = kT_j-block-stationary @ qT,
     exact-causal (q in [j*128, T) per k-block j). exp(scale*s) runs on
     ScalarE straight out of PSUM into a CAUSAL-PACKED bf16 wT tile.
     No max-subtraction: |scale*s| <= ~7 here, safely in f32/bf16 range.
     Diagonal 128x128 blocks additionally get a multiplicative 0/1
     strictly-causal mask into separate diag tiles.
  C) out[q,h] accumulates over k-blocks j<=i with wT blocks stationary and
     v_aug moving (N=129). Column 128 of PSUM is l = sum_k w; normalize
     with one reciprocal + ScalarE copy-with-per-partition-scale, DMA out.

Input streaming (the v1 kernel lost ~7 us here): per-DMA throughput caps
well below the ~358 GB/s aggregate (full-width chunk solo ~92 GB/s), and
concurrent DMAs round-robin at packet granularity so with all 8 chunk
DMAs in flight chunk 0's completion fires only after ~60% of the input
transferred. v4: every chunk is partition-split into sub-DMAs across the
SP+ACT trigger rings, ~3 chunks in flight via Pool WAR-link chaining, so
completions arrive in consumption order at full aggregate bandwidth.

PE warm-up: HAM's clock governor ramps with sustained engine density
(~1.2 GHz at start, full clock after ~3.4 us of UNBROKEN work; gaps delay
the ramp and the whole kernel runs slow). Dummy matmuls bridge the
launch/DMA dead window so the PE never idles from first dispatch on.

Engine discipline: hardware compute instructions carry at most ONE
semaphore wait (bacc's legalization splits the rest into event-semaphore
junctions; redundant same-engine self-waits are stripped post-build —
PE/ACT/DVE complete strictly in order). Tile tracks dependencies at
subtile granularity. exp and the final per-row scale run on ScalarE;
per-partition-scalar multiplies must use ScalarE activation scale APs
(DVE tensor_scalar AP / stride-0 broadcasts give wrong results on
hardware). The A->B junction interleaves q/k stop-matmuls with half-tile
PSUM->SBUF copies (qT halves on DVE, kT halves on ACT) so the first score
matmul's deps are ready the cycle the last stop-matmul retires.
"""

from contextlib import ExitStack

import ml_dtypes
import numpy as np

B, T, D, H = 8, 2048, 1024, 128
P = 128
DC = D // P  # 8 contraction chunks
TB = T // P  # 16 token blocks
QG = T // 512  # 4 512-wide token groups
SCALE = 1.0 / float(np.sqrt(H))

_CACHE = {}
LAST_RESULT = None


def _build():
    import concourse.bacc as bacc
    import concourse.mybir as mybir
    import concourse.tile as tile

    f32 = mybir.dt.float32
    bf16 = mybir.dt.bfloat16
    EXP = mybir.ActivationFunctionType.Exp
    MULT = mybir.AluOpType.mult
    ADD = mybir.AluOpType.add

    nc = bacc.Bacc()
    xT_h = nc.declare_dram_parameter("xT", [D, T], bf16, isOutput=False)
    # weights host-prelayouted to [p, c, h] so the DMA is contiguous
    # (2048 B per partition row; the [D, H] layout gave 256 B strided rows
    # below the 512 B DMA granule -> RMW-penalized, hogging the input queue)
    wq_h = nc.declare_dram_parameter("Wq", [P, DC, H], bf16, isOutput=False)
    wk_h = nc.declare_dram_parameter("Wk", [P, DC, H], bf16, isOutput=False)
    wv_h = nc.declare_dram_parameter("Wv", [P, DC, H], bf16, isOutput=False)
    mask_h = nc.declare_dram_parameter("mask", [P, P], bf16, isOutput=False)
    out_h = nc.declare_dram_parameter("out", [T, H], f32, isOutput=True)

    with tile.TileContext(nc) as tc:
        with ExitStack() as ctx:
            singles = ctx.enter_context(tc.tile_pool(name="singles", bufs=1))

            xT_sb = singles.tile([P, DC, T], bf16)
            wq_sb = singles.tile([P, DC, H], bf16)
            wk_sb = singles.tile([P, DC, H], bf16)
            wv_sb = singles.tile([P, DC, H], bf16)
            mask_sb = singles.tile([P, P], bf16)
            mask2_sb = singles.tile([P, P], bf16)
            qT_sb = singles.tile([P, T], bf16)
            kT_sb = singles.tile([P, T], bf16)
            v_sb = singles.tile([P, TB, 132], bf16)  # [...,128] = ones col
            wT_sb = singles.tile([P, 17408], bf16)  # causal-packed
            dw_sb = singles.tile([P, TB, P], bf16)  # masked diagonal blocks
            link_sb = singles.tile([P, DC + 2], bf16)  # DMA-chain dummies
            warm_sb = singles.tile([P, 512], bf16)
            # per-iteration epilogue slices (no pool recycling -> no WAR waits)
            rec_all = singles.tile([P, TB], f32)
            os_all = singles.tile([P, TB, P], f32)  # unnormalized staging
            ot_all = singles.tile([P, TB, P], f32)

            # Pool engine starts earliest (~6.0 us; its preamble memsets are
            # the first real ops in the v1 trace) — it seeds the warm-up
            # fodder the PE's dummy matmuls read.
            nc.gpsimd.memset(warm_sb, 0.0)
            nc.vector.memset(v_sb[:, :, 128:129], 1.0)

            # --- Input stream. Measured DMA law (see transcript probes): a
            # single full-width [128, 2048] DMA sustains only ~92 GB/s; 4
            # partition-subs of one chunk ~143; 4 concurrent full chunks
            # ~278; 8 concurrent ~333 (aggregate cap ~358). Per-DMA
            # throughput is the wall, so EVERY chunk is split into
            # partition-subs, and ~3 chunks (6 subs) stay in flight: chunk c
            # (c>=3) WAR-waits a Pool "link" op that RAW-waits chunk c-3's
            # completion. Completions then arrive IN CONSUMPTION ORDER every
            # ~1.4 us (all-concurrent DMAs round-robin at packet granularity,
            # which lands every completion at ~60% of the whole transfer —
            # v1 idled the PE 7.5 us waiting for chunk 0). Chunk 0 gets 4
            # subs (~143 GB/s solo) since its completion opens phase A.
            xT_ap = xT_h[:]

            def chunk_dma(c, nsub):
                w = P // nsub
                for s in range(nsub):
                    eng = nc.sync if s % 2 == 0 else nc.scalar
                    eng.dma_start(
                        out=xT_sb[w * s : w * (s + 1), c, :],
                        in_=xT_ap[c * P + w * s : c * P + w * (s + 1), :],
                    )

            def link(read_done_ap, war_ap, li):
                # RAW on read_done_ap's producer; the next DMA (writing
                # war_ap's region) then WARs on this op -> its trigger fires
                # only after the producer's completion semaphore.
                nc.gpsimd.tensor_tensor(
                    link_sb[:, li : li + 1], read_done_ap, war_ap, ADD
                )

            # Chunk 0 as a 4-sub head start (~143 GB/s vs 92 full-width solo)
            # + Wq/Wk as 2-subs: phase A's gate lands ~13 us. Chunks 1-7 ride
            # FULL-WIDTH and UNCHAINED right behind: concurrent full-width
            # DMAs each sustain near their solo rate (aggregate ~330), so
            # completions arrive in trigger order every ~0.65 us — faster
            # than the PE's 1.76 us/chunk consumption. (Chained or
            # partition-split streams cap at ~210-240 GB/s aggregate; v4's
            # chained 2-sub pipeline ran a 2.3 us chunk pitch and starved
            # the PE through all of phase A.)
            chunk_dma(0, 4)
            nc.sync.dma_start(out=wq_sb[0:64], in_=wq_h[0:64])
            nc.scalar.dma_start(out=wq_sb[64:128], in_=wq_h[64:128])
            nc.sync.dma_start(out=wk_sb[0:64], in_=wk_h[0:64])
            nc.scalar.dma_start(out=wk_sb[64:128], in_=wk_h[64:128])
            for c in range(1, DC):
                eng = nc.sync if c % 2 == 1 else nc.scalar
                eng.dma_start(out=xT_sb[:, c, :], in_=xT_ap[c * P : (c + 1) * P, :])
            # Phase-B-only inputs trail the stream (needed ~25 us) so they
            # don't steal chunk bandwidth.
            link(xT_sb[:, 5, T - 2 : T - 1], wv_sb[:, 0, 0:1], 0)
            nc.sync.dma_start(out=wv_sb, in_=wv_h[:])
            link(xT_sb[:, 6, T - 2 : T - 1], mask_sb[:, 0:1], 1)
            nc.scalar.dma_start(out=mask_sb, in_=mask_h[:])
            # ACT pre-touch: moves the mask's DMA wait onto a junction copy so
            # the per-j diag multiply's two deps (exp + mask) merge into one
            # ACT wait — hardware instructions carry at most one sem wait.
            nc.scalar.copy(mask2_sb, mask_sb)

            # --- Phase A1: q/k projections, d-chunk OUTER so each xT chunk is
            # consumed as its DMA lands (PE overlaps the input load). 8 PSUM
            # banks live at once; pool scoped so phase B/C reuse the space.
            with tc.tile_pool(name="psQK", bufs=1, space="PSUM") as psQK:
                # ONE persistent PSUM pool, SIX tiles, for the WHOLE
                # kernel: closing the A-pool and opening B-pools put a
                # release->alloc barrier (gated on ALL of A's copies) before
                # the first score matmul — ~1.2 us. Phase B's buffers ARE
                # phase A's accumulators with clean per-TILE dependencies:
                #   S0/S1 [P,1024]: q accum (A) -> score ping-pong (B)
                #   kps[0] [P,512]: k group 0 (A) -> v-projection (B)
                #   kps[1..3]:      k groups 1-3 (A) -> 3 output slots (B)
                S0 = psQK.tile([P, 1024], f32, tag="S0")
                S1 = psQK.tile([P, 1024], f32, tag="S1")
                kps = [
                    psQK.tile([P, 512], f32, tag=f"kps{g}", name=f"kps{g}")
                    for g in range(QG)
                ]
                # PE warm-up fodder: the PE sequencer starts ~8.1 us (its
                # ~770-instruction queue loads last); 12 x 512 dummies end
                # ~13.2 us, dovetailing with chunk 0's completion. HAM's
                # clock governor ramps with engine DENSITY — v3's sparse
                # early stream delayed full clock to 25.7 us and the WHOLE
                # kernel ran ~15% slow — so the PE must never idle from
                # warm-up start onward. They write qps[0] BEFORE its real
                # accumulation group begins (start=True clears the bank).
                for _ in range(13):
                    nc.tensor.matmul(
                        S0[:, 0:512], warm_sb[:, 0:128], warm_sb,
                        start=True, stop=True,
                    )

                def qk_mm(is_q, g, c):
                    w_sb = wq_sb if is_q else wk_sb
                    if is_q:
                        acc = (S0 if g < 2 else S1)[
                            :, (g % 2) * 512 : (g % 2) * 512 + 512
                        ]
                    else:
                        acc = kps[g]
                    nc.tensor.matmul(
                        acc,
                        w_sb[:, c, :],
                        xT_sb[:, c, g * 512 : (g + 1) * 512],
                        start=(c == 0),
                        stop=(c == DC - 1),
                    )

                for c in range(DC - 1):
                    for is_q in (True, False):
                        for g in range(QG):
                            qk_mm(is_q, g, c)
                # Last chunk: stop-matmuls with their tiles' PSUM->SBUF
                # copies (qT halves on DVE after both their groups' stops,
                # kT 512s on ACT per group). Per-tile deps: no copy blocks
                # another tile's stop, and phase B's first writers wait only
                # these copies — stops, then scores, NO pool barrier.
                qk_mm(True, 0, DC - 1)
                qk_mm(True, 1, DC - 1)
                nc.vector.tensor_copy(qT_sb[:, 0:1024], S0)
                qk_mm(True, 2, DC - 1)
                qk_mm(True, 3, DC - 1)
                nc.vector.tensor_copy(qT_sb[:, 1024:2048], S1)
                for g in range(QG):
                    qk_mm(False, g, DC - 1)
                    nc.scalar.copy(
                        kT_sb[:, g * 512 : (g + 1) * 512], kps[g]
                    )

                # --- Phases B+A2+C, software-pipelined by one j: per k-block
                # j emit its score matmuls + exp + v projection, then output
                # group C_{j-1}, whose inputs (exps/dw/v for blocks <= j-1)
                # are all complete by then — so C's matmuls carry no waits and
                # the PE stream stays dense while ScalarE exps run alongside.
                out_ap = out_h[:]

                # Causal-packed wT layout: segment for k-block j holds
                # q in [j*128, T) at packed offset OFF[j]; segments are
                # back-to-back so exp runs in maximal 1024-wide ops across
                # block boundaries (ACT op overhead is ~352 cycles each).
                OFF = [0] * (TB + 1)
                for j in range(TB):
                    OFF[j + 1] = OFF[j] + (T - j * P)
                TOTAL = OFF[TB]  # 17408

                def wT_at(jj, qstart, width):
                    o = OFF[jj] + (qstart - jj * P)
                    return wT_sb[:, o : o + width]

                def emit_c_group(i):
                    po = kps[1 + i % 3][:, 0:132]
                    for jj in range(i):
                        nc.tensor.matmul(
                            po[:, 0:129],
                            wT_at(jj, i * P, P),
                            v_sb[:, jj, 0:129],
                            start=(jj == 0),
                            stop=False,
                        )
                    nc.tensor.matmul(
                        po[:, 0:129],
                        dw_sb[:, i, :],
                        v_sb[:, i, 0:129],
                        start=(i == 0),
                        stop=True,
                    )
                    nc.vector.reciprocal(rec_all[:, i : i + 1], po[:, 128:129])
                    # per-partition normalize OFF ScalarE: with it there,
                    # ACT's B-phase ledger (17 exps ~17us + 16 muls ~7us)
                    # exceeded the PE's ~22us window and became co-critical.
                    # DVE's tensor_scalar reads PSUM WRONG on hardware (rel
                    # err 19!) but is correct from SBUF (probe), and Pool's
                    # tensor_scalar takes 2us/op (slow DSP): so DVE copies
                    # PSUM->SBUF (its usual, correct path), then multiplies
                    # in SBUF, both on DVE.
                    # The LAST two groups normalize on ScalarE directly from
                    # PSUM (its exps are done by then; the activation-scale
                    # path is PSUM-correct) — one hop fewer on the tail.
                    if i >= TB - 2:
                        nc.scalar.mul(
                            ot_all[:, i, :], po[:, 0:H], rec_all[:, i : i + 1]
                        )
                    else:
                        nc.vector.tensor_copy(os_all[:, i, :], po[:, 0:H])
                        nc.vector.tensor_scalar_mul(
                            ot_all[:, i, :], os_all[:, i, :], rec_all[:, i : i + 1]
                        )
                    # last group's DMA triggers from the ACT ring — same
                    # engine as its normalize, no cross-engine hop.
                    deng = nc.scalar if i == TB - 1 else nc.sync
                    deng.dma_start(
                        out=out_ap[i * P : (i + 1) * P, :], in_=ot_all[:, i, :]
                    )

                def emit_j_epilogue(j):
                    # diag mask (on Pool — all-SBUF, keeps DVE for the
                    # PSUM-side copies), v projection, pipelined output
                    # group. The LAST diag multiply sits on the kernel's
                    # critical tail: DVE's 190 ns beats Pool's 405 ns there.
                    eng = nc.vector if j == TB - 1 else nc.gpsimd
                    eng.tensor_tensor(
                        dw_sb[:, j, :], wT_at(j, j * P, P), mask2_sb, MULT
                    )
                    pv = kps[0][:, 0:H]
                    for c in range(DC):
                        nc.tensor.matmul(
                            pv,
                            xT_sb[:, c, j * P : (j + 1) * P],
                            wv_sb[:, c, :],
                            start=(c == 0),
                            stop=(c == DC - 1),
                        )
                    nc.vector.tensor_copy(v_sb[:, j, 0:H], pv)
                    if j > 0:
                        emit_c_group(j - 1)

                # 1024-wide exp chunks amortize ACT's ~352-cycle fixed cost,
                # but the tail goes FINER: segs 13/14/15's diag heads all sit
                # in the last 1024 cols, so one last big exp would serialize
                # the last three output groups (~58 matmuls) behind it. Tail
                # chunks split exactly AT segment boundaries, so the final
                # exp is ONLY seg15's 128-col head — everything else the
                # last output group reads is exp'd before it.
                chunk_bounds = [(t, 1024) for t in range(0, 16384, 1024)]
                chunk_bounds += [(16384, 256), (16640, 384), (17024, 256), (17280, 128)]
                next_done = 0  # next j whose epilogue is pending
                for ci, (ts, tw) in enumerate(chunk_bounds):
                    ps = S0 if ci % 2 == 0 else S1
                    # score matmuls covering packed [ts, ts+tw): split at the
                    # PSUM bank boundary (ts+512) and at segment boundaries
                    for j in range(TB):
                        lo = max(ts, OFF[j])
                        hi = min(ts + tw, OFF[j + 1])
                        a = lo
                        while a < hi:
                            bank_end = ts + 512 if a < ts + 512 else ts + 1024
                            b = min(hi, bank_end)
                            qg = j * P + (a - OFF[j])
                            nc.tensor.matmul(
                                ps[:, a - ts : b - ts],
                                kT_sb[:, j * P : (j + 1) * P],
                                qT_sb[:, qg : qg + (b - a)],
                                start=True,
                                stop=True,
                            )
                            a = b
                    nc.scalar.activation(
                        wT_sb[:, ts : ts + tw], ps[:, :tw], EXP, scale=SCALE
                    )
                    # epilogue j needs only segment j's first 128 cols exp'd
                    # (dw_j's diag region; C_{j-1}'s deepest read is shallower)
                    while next_done < TB and OFF[next_done] + P <= ts + tw:
                        emit_j_epilogue(next_done)
                        next_done += 1
                emit_c_group(TB - 1)

    _strip_self_waits(nc)
    nc.finalize()  # Bacc.compile(): wait legalization + register allocation
    return nc


def _strip_self_waits(nc):
    """Drop same-engine semaphore waits on in-order engines (PE/ACT/DVE
    execute and complete strictly in order, so a self-wait is redundant).
    Tile emits them conservatively; walrus allows only one sem wait per
    compute instruction, and these push some matmuls/tensor-ops over."""
    prefixes = {"PE": "PE_", "Activation": "Activation_", "DVE": "DVE_"}
    for bb in nc.m.functions[0].blocks:
        for inst in bb.instructions:
            si = inst.sync_info
            if not si or not si.on_wait:
                continue
            pref = prefixes.get(str(inst.engine).split(".")[-1])
            if pref is None:
                continue
            keep = [w for w in si.on_wait if not (w.ant_name or "").startswith(pref)]
            if len(keep) != len(si.on_wait):
                si.on_wait = keep
                inst.sync_info = si


def kernel(**inputs):
    global LAST_RESULT
    x = np.asarray(inputs["x"], dtype=np.float32)
    bf = ml_dtypes.bfloat16
    w_bf = {
        k: np.ascontiguousarray(
            np.asarray(inputs[k], dtype=np.float32)
            .astype(bf)
            .reshape(DC, P, H)
            .transpose(1, 0, 2)
        )
        for k in ("Wq", "Wk", "Wv")
    }
    # dw[p=k_local, f=q_local] keeps entries with k <= q
    mask01 = (
        (np.arange(P)[:, None] <= np.arange(P)[None, :]).astype(np.float32).astype(bf)
    )

    if "nc" not in _CACHE:
        _CACHE["nc"] = _build()
    nc = _CACHE["nc"]

    from concourse.bass_utils import run_bass_kernel_spmd

    in_maps = [
        {
            "xT": np.ascontiguousarray(x[b].T).astype(bf),
            "Wq": w_bf["Wq"],
            "Wk": w_bf["Wk"],
            "Wv": w_bf["Wv"],
            "mask": mask01,
        }
        for b in range(B)
    ]
    res = run_bass_kernel_spmd(nc, in_maps, core_ids=list(range(B)))
    LAST_RESULT = res
    return np.stack([res.results[b]["out"] for b in range(B)]).astype(np.float32)
